# revision 32
# baseline (speedup 1.0000x reference)
"""Trainium2 Bass kernel for DecomposedQValueNN (gnn_message_passing).

Per batch row b of x[65536, 128]:
  xa = x.reshape(B, 32, 4); other_a = MLP_o(xa[:,a]) (3 relu layers, 4-32-32-16)
  sum_other = sum_{a != sel} other_a;  sel_out = MLP_s(xa[:,sel])
  h = relu([sel_out; sum_other] @ gW1 + gb1); q = h @ gW2 + gb2
  out[b] = q[b, clip(int(xa[b,sel,3]),0,1)]

V3 design (8 cores, batch data-parallel, 8192 rows/core):
  - host transposes + bf16-casts x to [feat=128, rows]
  - ALL layer matmuls use full K=128 contraction (block-diagonal packed
    weights), so only ~33 matmul instructions per 512-row chunk:
      L1: 8 MMs, variant u covers agents {u,8+u,16+u,24+u}
          (W1u[32k+4u+d, 32k+h] = oW1[d,h]) -> z1u [128=4ag x 32hid, 512]
      L2: 8 MMs, shared block-diag W2 -> z2u [128, 512]
      L3: 8 MMs, W3 block-diag [128, 64]; dual-pair PSUM tile [128,1024]
          packs four u-variants -> z3 [128 = 8ag x 16, 512] per pair
      gsum: 4 accumulating MMs (K=128) against replicated gW1[16:32]
          rows (sel agent's 16-row stripe zeroed in one variant)
      + gsel (K=16), qp, batched sel-MLP (3 MMs/chunk amortized)
  - PSUM->SBUF relu evacuations alternate scalar/vector engines over
    [128,1024] pair tiles (GPSIMD cannot read PSUM on TRN2)
  - global head batched per 4-chunk group: zg PSUM tile [128,512] holds
    4 chunks' [32,512] stripes; one hg evac + 4 qp MMs + one q evac per
    group; final q staged in SBUF and DMA'd out with per-stripe DMAs
  - software pipelining: tail (gsum chain) of chunk c-4 interleaved
    between chunk c's layer blocks; remaining 4 tails drain col-parallel
  Final 2-way q gather on host.
"""

import numpy as np
import ml_dtypes

BF16 = ml_dtypes.bfloat16

B_FULL = 65536
N_CORES = 8
B_C = B_FULL // N_CORES       # 8192
A, D = 32, 4
NCH = 512                     # batch cols per PSUM bank (fp32)
CHUNKS = B_C // NCH           # 16

# wpack (bf16) column offsets
OW1 = 0            # 8 x [128, 128] L1 block-diag variants u=0..7
OW2 = 1024         # [128, 128] block-diag (4 copies of oW2)
OW3 = 1152         # [128, 64]  block-diag (4 copies of oW3 -> 16-dim)
OSW1 = 1216        # [128, 32]
OSW2 = 1248
OSW3 = 1280
OGSUMF = 1312      # [128, 32] gW1[16+(p%16)] at every partition
OGSUME = 1344      # same, sel agent's 16-row stripe zeroed
OGSEL = 1376       # [128, 32] rows 32c..32c+16 = gW1[0:16]
OGW2 = 1408        # [128, 32] gW2 padded, tiled on all 4 row groups
WCOLS = 1440

_COMPILED = {}
LAST_RESULT = None


def _f32(a):
    return np.ascontiguousarray(a, dtype=np.float32)


def _build_wpack(sel, oW1, oW2, oW3, sW1, sW2, sW3, gW1, gW2):
    P = 128
    oW1, oW2, oW3 = _f32(oW1), _f32(oW2), _f32(oW3)

    # L1 variants: W1u[32k+4u+d, 32k+h] = oW1[d, h]
    w1 = np.zeros((P, 8 * P), np.float32)
    for u in range(8):
        for k in range(4):
            w1[32 * k + 4 * u:32 * k + 4 * u + 4,
               128 * u + 32 * k:128 * u + 32 * k + 32] = oW1
    # L2 block-diag: 4 copies of oW2
    w2 = np.zeros((P, P), np.float32)
    for k in range(4):
        w2[32 * k:32 * k + 32, 32 * k:32 * k + 32] = oW2
    # L3 block-diag: W3[32k+h, 16k+t] = oW3[h, t]
    w3 = np.zeros((P, 64), np.float32)
    for k in range(4):
        w3[32 * k:32 * k + 32, 16 * k:16 * k + 16] = oW3

    u_sel = sel % 8
    sw1p = np.zeros((32, 32), np.float32)
    sw1p[4 * u_sel:4 * u_sel + 4, :] = _f32(sW1)
    sw1 = np.tile(sw1p, (4, 1))
    sw2 = np.tile(_f32(sW2), (4, 1))
    sW3p = _f32(sW3)
    sW3p = np.concatenate([sW3p, np.zeros((32, 16), np.float32)], 1)
    sw3 = np.tile(sW3p, (4, 1))

    # z3 layout: partition 64c + 16k + t  <->  agent 8k + (2g + c), out t
    g = _f32(gW1)
    gsumf = np.zeros((P, 32), np.float32)
    for p in range(P):
        gsumf[p, :] = g[16 + (p % 16), :]
    gsume = gsumf.copy()
    c_sel = u_sel % 2
    k_sel = sel // 8
    off = 64 * c_sel + 16 * k_sel
    gsume[off:off + 16, :] = 0.0

    # gsel replicated: rows 32c+m (m<16) = gW1[m] (batched sel tile stripes)
    gselw = np.zeros((P, 32), np.float32)
    for c in range(4):
        gselw[32 * c:32 * c + 16, :] = g[0:16, :]
    gw2p = np.zeros((32, 32), np.float32)
    gw2p[:, 0:2] = _f32(gW2)
    gw2 = np.tile(gw2p, (4, 1))

    parts = [w1, w2, w3, sw1, sw2, sw3, gsumf, gsume, gselw, gw2]
    wp = np.concatenate(parts, axis=1)
    assert wp.shape == (P, WCOLS), wp.shape
    return np.ascontiguousarray(wp.astype(BF16))


def _build_bias(ob1, ob2, ob3, sb1, sb2, sb3, gb1, gb2):
    # fp32 per-partition bias columns: col k pattern = b[(p % 32) % len]
    P = 128

    def bias_col(b, valid=32):
        v = np.zeros((P, 1), np.float32)
        b = _f32(b).ravel()
        for p in range(P):
            r = p % 32
            if r < valid:
                v[p, 0] = b[r % len(b)]
        return v

    cols = [bias_col(ob1), bias_col(ob2), bias_col(ob3, 32),
            bias_col(sb1), bias_col(sb2), bias_col(sb3, 16),
            bias_col(gb1), bias_col(gb2, 2)]
    return np.ascontiguousarray(np.concatenate(cols, 1))  # [128, 8] fp32


BB1, BB2, BB3, BSB1, BSB2, BSB3, BGB1, BGB2 = range(8)


def _build_nc(sel):
    import concourse.bacc as bacc
    import concourse.mybir as mybir
    from concourse.tile import TileContext

    f32 = mybir.dt.float32
    bf16 = mybir.dt.bfloat16
    Relu = mybir.ActivationFunctionType.Relu
    Ident = mybir.ActivationFunctionType.Identity
    add_op = mybir.AluOpType.add
    max_op = mybir.AluOpType.max

    u_sel = sel % 8
    g_sel = u_sel // 2          # z3 pair holding the sel agent
    si = sel // 8               # partition group of sel agent's features

    nc = bacc.Bacc("TRN2", target_bir_lowering=False, debug=False,
                   num_devices=N_CORES)
    x_ext = nc.dram_tensor("xt", [128, B_C], bf16, kind="ExternalInput").ap()
    w_ext = nc.dram_tensor("wpack", [128, WCOLS], bf16, kind="ExternalInput").ap()
    b_ext = nc.dram_tensor("bias", [128, 8], f32, kind="ExternalInput").ap()
    o_ext = nc.dram_tensor("out", [2, B_C], f32, kind="ExternalOutput").ap()

    with TileContext(nc) as tc:
        with (
            tc.tile_pool(name="const", bufs=1) as cpool,
            tc.tile_pool(name="xin", bufs=8) as xpool,
            tc.tile_pool(name="h", bufs=6) as hpool,
            tc.tile_pool(name="z3s", bufs=20) as z3pool,
            tc.tile_pool(name="gl", bufs=2) as glpool,
            tc.tile_pool(name="qs", bufs=4) as qpool,
            tc.tile_pool(name="zp", bufs=2, space="PSUM") as zpool,
            tc.tile_pool(name="z3p", bufs=2, space="PSUM") as z3ppool,
            tc.tile_pool(name="gz", bufs=1, space="PSUM") as gzpool,
            tc.tile_pool(name="gm", bufs=1, space="PSUM") as gmpool,
        ):
            W = cpool.tile([128, WCOLS], bf16, name="W")
            nc.sync.dma_start(out=W[:], in_=w_ext[:])
            BI = cpool.tile([128, 8], f32, name="BI")
            nc.sync.dma_start(out=BI[:], in_=b_ext[:])

            ECYC = [0]

            def evac(dst, src, bcol, func, lo=0, size=128, eng=None):
                b = BI[lo:lo + size, bcol:bcol + 1]
                if eng is None:
                    eng = ECYC[0] % 2
                    ECYC[0] += 1
                if eng == 0:
                    if func == "relu":
                        nc.scalar.activation(dst, src, Relu, bias=b)
                    else:
                        nc.scalar.activation(dst, src, Ident, bias=b)
                else:
                    if func == "relu":
                        nc.vector.tensor_scalar(dst, src, b, 0.0, add_op, max_op)
                    else:
                        nc.vector.tensor_scalar_add(dst, src, b)

            xin_t = {}
            h1_t = {}
            h2_t = {}
            z3s_all = {}
            bsh3_g = {}
            zg_t = {}
            qsb_t = {}

            def emit_xin(c):
                b0 = c * NCH
                xin = xpool.tile([128, NCH], bf16, tag="xin", name=f"xin{c}")
                nc.sync.dma_start(out=xin[:], in_=x_ext[:, b0:b0 + NCH])
                xin_t[c] = xin

            def emit_L1_pair(c, p):
                xin = xin_t[c]
                z1 = zpool.tile([128, 2 * NCH], f32, tag="z", name=f"z1_{c}_{p}")
                for h in range(2):
                    u = 2 * p + h
                    nc.tensor.matmul(
                        z1[:, NCH * h:NCH * h + NCH],
                        W[:, OW1 + 128 * u:OW1 + 128 * u + 128],
                        xin[:], start=True, stop=True)
                h1 = hpool.tile([128, 2 * NCH], bf16, tag="h1", name=f"h1_{c}_{p}")
                evac(h1[:, 0:NCH], z1[:, 0:NCH], BB1, "relu", eng=0)
                evac(h1[:, NCH:2 * NCH], z1[:, NCH:2 * NCH], BB1, "relu",
                     eng=1)
                h1_t[(c, p)] = h1

            def emit_L2_pair(c, p):
                h1 = h1_t.pop((c, p))
                z2 = zpool.tile([128, 2 * NCH], f32, tag="z", name=f"z2_{c}_{p}")
                for h in range(2):
                    nc.tensor.matmul(
                        z2[:, NCH * h:NCH * h + NCH],
                        W[:, OW2:OW2 + 128],
                        h1[:, NCH * h:NCH * h + NCH], start=True, stop=True)
                h2 = hpool.tile([128, 2 * NCH], bf16, tag="h2", name=f"h2_{c}_{p}")
                evac(h2[:], z2[:], BB2, "relu", eng=(1 if p == 3 else 0))
                h2_t[(c, p)] = h2

            def emit_L3_pair(c, p):
                # z3 pair tile: part 64c' + 16k + t <-> agent 8k + 2p + c'
                z3 = z3ppool.tile([128, NCH], f32, tag="z3",
                                  name=f"z3_{c}_{p}")
                hp = h2_t[(c, p)]
                for h in range(2):
                    nc.tensor.matmul(
                        z3[64 * h:64 * h + 64, :],
                        W[:, OW3:OW3 + 64],
                        hp[:, NCH * h:NCH * h + NCH],
                        start=True, stop=True)
                del h2_t[(c, p)]
                z3sb = z3pool.tile([128, NCH], bf16, tag="z3s",
                                   name=f"z3s_{c}_{p}")
                evac(z3sb[:], z3[:], BB3, "relu", eng=1)
                z3s_all.setdefault(c, {})[p] = z3sb

            def emit_selb(grp):
                # batched sel-MLP for chunks 4g..4g+3: chunk stripe = 32*(c%4)
                bz1 = gmpool.tile([128, NCH], f32, tag="g", name=f"bz1_{grp}")
                for cl in range(4):
                    cc = 4 * grp + cl
                    nc.tensor.matmul(
                        bz1[32 * cl:32 * cl + 32, :],
                        W[32 * si:32 * si + 32, OSW1:OSW1 + 32],
                        xin_t[cc][32 * si:32 * si + 32, :],
                        start=True, stop=True,
                        tile_position=(32 * si, 32 * cl))
                bsh1 = glpool.tile([128, NCH], bf16, tag="sh1", name=f"bsh1_{grp}")
                evac(bsh1[:], bz1[:], BSB1, "relu", eng=0)
                bz2 = gmpool.tile([128, NCH], f32, tag="g", name=f"bz2_{grp}")
                for cl in range(4):
                    nc.tensor.matmul(
                        bz2[32 * cl:32 * cl + 32, :],
                        W[32 * cl:32 * cl + 32, OSW2:OSW2 + 32],
                        bsh1[32 * cl:32 * cl + 32, :],
                        start=True, stop=True,
                        tile_position=(32 * cl, 32 * cl))
                bsh2 = glpool.tile([128, NCH], bf16, tag="sh2", name=f"bsh2_{grp}")
                evac(bsh2[:], bz2[:], BSB2, "relu", eng=0)
                bz3 = gmpool.tile([128, NCH], f32, tag="g", name=f"bz3_{grp}")
                for cl in range(4):
                    nc.tensor.matmul(
                        bz3[32 * cl:32 * cl + 32, :],
                        W[32 * cl:32 * cl + 32, OSW3:OSW3 + 32],
                        bsh2[32 * cl:32 * cl + 32, :],
                        start=True, stop=True,
                        tile_position=(32 * cl, 32 * cl))
                bsh3 = glpool.tile([128, NCH], bf16, tag="sh3", name=f"bsh3_{grp}")
                evac(bsh3[:], bz3[:], BSB3, "relu", eng=0)
                bsh3_g[grp] = bsh3

            def emit_gsum_mm(c, step):
                # stripe cl = c%4 of the group tile zg_{c//4}
                G, cl = c // 4, c % 4
                if step == 0 and cl == 0:
                    zg_t[G] = gzpool.tile([128, NCH], f32, tag="zg",
                                          name=f"zg_{G}")
                zg = zg_t[G]
                excl = (step == g_sel)
                oo = OGSUME if excl else OGSUMF
                nc.tensor.matmul(
                    zg[32 * cl:32 * cl + 32, :], W[:, oo:oo + 32],
                    z3s_all[c][step][:, :],
                    start=(step == 0), stop=False,
                    tile_position=(0, 32 * cl))

            def emit_gsel_mm(c):
                G, cl = c // 4, c % 4
                bsh3 = bsh3_g[c // 4]
                nc.tensor.matmul(
                    zg_t[G][32 * cl:32 * cl + 32, :],
                    W[32 * cl:32 * cl + 16, OGSEL:OGSEL + 32],
                    bsh3[32 * cl:32 * cl + 16, :],
                    start=False, stop=True, tile_position=(32 * cl, 32 * cl))
                del z3s_all[c]

            def emit_group_fin(G):
                # all 4 stripes of zg_G accumulated: one hg evac, 4 qp MMs,
                # one q evac into SBUF staging, 4 output DMAs
                zg = zg_t.pop(G)
                hg = glpool.tile([128, NCH], bf16, tag="hg", name=f"hg_{G}")
                evac(hg[:], zg[:], BGB1, "relu", eng=0)
                qp = gmpool.tile([128, NCH], f32, tag="g", name=f"qp_{G}")
                for cl in range(4):
                    nc.tensor.matmul(
                        qp[32 * cl:32 * cl + 32, :],
                        W[32 * cl:32 * cl + 32, OGW2:OGW2 + 32],
                        hg[32 * cl:32 * cl + 32, :],
                        start=True, stop=True,
                        tile_position=(32 * cl, 32 * cl))
                qsb = qpool.tile([128, NCH], f32, tag="q", name=f"qsb_{G}")
                evac(qsb[:], qp[:], BGB2, "add", eng=0)
                qsb_t[G] = qsb
                for cl in range(4):
                    b0 = (4 * G + cl) * NCH
                    nc.sync.dma_start(
                        out=o_ext[0:2, b0:b0 + NCH],
                        in_=qsb[32 * cl:32 * cl + 2, :])

            LAG = 4  # tail(c) needs bsh3 of group c//4 (ready in chunk 4g+3)
            for c in range(2):
                emit_xin(c)
            for c in range(CHUNKS):
                if c + 2 < CHUNKS:
                    emit_xin(c + 2)
                tail = c - LAG if c >= LAG else None
                emit_L1_pair(c, 0)
                emit_L1_pair(c, 1)
                if tail is not None:
                    emit_gsum_mm(tail, 0)
                emit_L1_pair(c, 2)
                emit_L1_pair(c, 3)
                if tail is not None:
                    emit_gsum_mm(tail, 1)
                emit_L2_pair(c, 0)
                emit_L2_pair(c, 1)
                if tail is not None:
                    emit_gsum_mm(tail, 2)
                emit_L2_pair(c, 2)
                emit_L2_pair(c, 3)
                if tail is not None:
                    emit_gsum_mm(tail, 3)
                    emit_gsel_mm(tail)
                emit_L3_pair(c, 0)
                emit_L3_pair(c, 1)
                if tail is not None and tail % 4 == 3:
                    emit_group_fin(tail // 4)
                emit_L3_pair(c, 2)
                emit_L3_pair(c, 3)
                if c % 4 == 3:
                    emit_selb(c // 4)
            # drain: remaining LAG tails (one full group), stripes
            # col-parallel so the 4 accumulation chains interleave on the PE
            t0 = CHUNKS - LAG
            for st in range(4):
                for t in range(t0, CHUNKS):
                    emit_gsum_mm(t, st)
            for t in range(t0, CHUNKS):
                emit_gsel_mm(t)
            emit_group_fin(t0 // 4)
    nc.compile()
    return nc


def kernel(**inputs):
    x = _f32(inputs["joint_state_actions"])
    sel = int(inputs["selected_agent_idx"])

    wpack = _build_wpack(
        sel, inputs["oW1"], inputs["oW2"], inputs["oW3"],
        inputs["sW1"], inputs["sW2"], inputs["sW3"],
        inputs["gW1"], inputs["gW2"])
    bias = _build_bias(
        inputs["ob1"], inputs["ob2"], inputs["ob3"],
        inputs["sb1"], inputs["sb2"], inputs["sb3"],
        inputs["gb1"], inputs["gb2"])

    if sel not in _COMPILED:
        _COMPILED[sel] = _build_nc(sel)
    nc = _COMPILED[sel]

    from concourse.bass_utils import run_bass_kernel_spmd
    shards = [np.ascontiguousarray(x[i * B_C:(i + 1) * B_C].T.astype(BF16))
              for i in range(N_CORES)]
    in_maps = [{"xt": s, "wpack": wpack, "bias": bias} for s in shards]
    import os
    trace = bool(int(os.environ.get("KERNEL_TRACE", "0")))
    res = run_bass_kernel_spmd(nc, in_maps, list(range(N_CORES)),
                               trace=trace)
    global LAST_RESULT
    LAST_RESULT = res

    q01 = np.concatenate([res.results[i]["out"] for i in range(N_CORES)],
                         axis=1)
    act = np.clip(x[:, 4 * sel + 3].astype(np.int32), 0, 1)
    out = np.where(act == 0, q01[0], q01[1]).astype(np.float32)
    return out[:, None]


# revision 33
# speedup vs baseline: 1.1366x; 1.1366x over previous
"""Trainium2 Bass kernel for DecomposedQValueNN (gnn_message_passing).

Per batch row b of x[65536, 128]:
  xa = x.reshape(B, 32, 4); other_a = MLP_o(xa[:,a]) (3 relu layers, 4-32-32-16)
  sum_other = sum_{a != sel} other_a;  sel_out = MLP_s(xa[:,sel])
  h = relu([sel_out; sum_other] @ gW1 + gb1); q = h @ gW2 + gb2
  out[b] = q[b, clip(int(xa[b,sel,3]),0,1)]

V3 design (8 cores, batch data-parallel, 8192 rows/core):
  - host transposes + bf16-casts x to [feat=128, rows]
  - ALL layer matmuls use full K=128 contraction (block-diagonal packed
    weights), so only ~33 matmul instructions per 512-row chunk:
      L1: 8 MMs, variant u covers agents {u,8+u,16+u,24+u}
          (W1u[32k+4u+d, 32k+h] = oW1[d,h]) -> z1u [128=4ag x 32hid, 512]
      L2: 8 MMs, shared block-diag W2 -> z2u [128, 512]
      L3: 8 MMs, W3 block-diag [128, 64]; dual-pair PSUM tile [128,1024]
          packs four u-variants -> z3 [128 = 8ag x 16, 512] per pair
      gsum: 4 accumulating MMs (K=128) against replicated gW1[16:32]
          rows (sel agent's 16-row stripe zeroed in one variant)
      + gsel (K=16), qp, batched sel-MLP (3 MMs/chunk amortized)
  - PSUM->SBUF relu evacuations alternate scalar/vector engines over
    [128,1024] pair tiles (GPSIMD cannot read PSUM on TRN2)
  - global head batched per 4-chunk group: zg PSUM tile [128,512] holds
    4 chunks' [32,512] stripes; one hg evac + 4 qp MMs + one q evac per
    group; final q staged in SBUF and DMA'd out with per-stripe DMAs
  - software pipelining: tail (gsum chain) of chunk c-4 interleaved
    between chunk c's layer blocks; remaining 4 tails drain col-parallel
  Final 2-way q gather on host.
"""

import numpy as np
import ml_dtypes

BF16 = ml_dtypes.bfloat16

B_FULL = 65536
N_CORES = 8
B_C = B_FULL // N_CORES       # 8192
A, D = 32, 4
NCH = 512                     # batch cols per PSUM bank (fp32)
CHUNKS = B_C // NCH           # 16

# wpack (bf16) column offsets
OW1 = 0            # 8 x [128, 128] L1 block-diag variants u=0..7
OW2 = 1024         # [128, 128] block-diag (4 copies of oW2)
OW3 = 1152         # [128, 64]  block-diag (4 copies of oW3 -> 16-dim)
OSW1 = 1216        # [128, 32]
OSW2 = 1248
OSW3 = 1280
OGSUMF = 1312      # [128, 32] gW1[16+(p%16)] at every partition
OGSUME = 1344      # same, sel agent's 16-row stripe zeroed
OGSEL = 1376       # [128, 32] rows 32c..32c+16 = gW1[0:16]
OGW2 = 1408        # [128, 32] gW2 padded, tiled on all 4 row groups
WCOLS = 1440

_COMPILED = {}
LAST_RESULT = None


def _f32(a):
    return np.ascontiguousarray(a, dtype=np.float32)


def _build_wpack(sel, oW1, oW2, oW3, sW1, sW2, sW3, gW1, gW2):
    P = 128
    oW1, oW2, oW3 = _f32(oW1), _f32(oW2), _f32(oW3)

    # L1 variants: W1u[32k+4u+d, 32k+h] = oW1[d, h]
    w1 = np.zeros((P, 8 * P), np.float32)
    for u in range(8):
        for k in range(4):
            w1[32 * k + 4 * u:32 * k + 4 * u + 4,
               128 * u + 32 * k:128 * u + 32 * k + 32] = oW1
    # L2 block-diag: 4 copies of oW2
    w2 = np.zeros((P, P), np.float32)
    for k in range(4):
        w2[32 * k:32 * k + 32, 32 * k:32 * k + 32] = oW2
    # L3 block-diag: W3[32k+h, 16k+t] = oW3[h, t]
    w3 = np.zeros((P, 64), np.float32)
    for k in range(4):
        w3[32 * k:32 * k + 32, 16 * k:16 * k + 16] = oW3

    u_sel = sel % 8
    sw1p = np.zeros((32, 32), np.float32)
    sw1p[4 * u_sel:4 * u_sel + 4, :] = _f32(sW1)
    sw1 = np.tile(sw1p, (4, 1))
    sw2 = np.tile(_f32(sW2), (4, 1))
    sW3p = _f32(sW3)
    sW3p = np.concatenate([sW3p, np.zeros((32, 16), np.float32)], 1)
    sw3 = np.tile(sW3p, (4, 1))

    # z3 layout: partition 64c + 16k + t  <->  agent 8k + (2g + c), out t
    g = _f32(gW1)
    gsumf = np.zeros((P, 32), np.float32)
    for p in range(P):
        gsumf[p, :] = g[16 + (p % 16), :]
    gsume = gsumf.copy()
    c_sel = u_sel % 2
    k_sel = sel // 8
    off = 64 * c_sel + 16 * k_sel
    gsume[off:off + 16, :] = 0.0

    # gsel replicated: rows 32c+m (m<16) = gW1[m] (batched sel tile stripes)
    gselw = np.zeros((P, 32), np.float32)
    for c in range(4):
        gselw[32 * c:32 * c + 16, :] = g[0:16, :]
    gw2p = np.zeros((32, 32), np.float32)
    gw2p[:, 0:2] = _f32(gW2)
    gw2 = np.tile(gw2p, (4, 1))

    parts = [w1, w2, w3, sw1, sw2, sw3, gsumf, gsume, gselw, gw2]
    wp = np.concatenate(parts, axis=1)
    assert wp.shape == (P, WCOLS), wp.shape
    return np.ascontiguousarray(wp.astype(BF16))


def _build_bias(ob1, ob2, ob3, sb1, sb2, sb3, gb1, gb2):
    # fp32 per-partition bias columns: col k pattern = b[(p % 32) % len]
    P = 128

    def bias_col(b, valid=32):
        v = np.zeros((P, 1), np.float32)
        b = _f32(b).ravel()
        for p in range(P):
            r = p % 32
            if r < valid:
                v[p, 0] = b[r % len(b)]
        return v

    cols = [bias_col(ob1), bias_col(ob2), bias_col(ob3, 32),
            bias_col(sb1), bias_col(sb2), bias_col(sb3, 16),
            bias_col(gb1), bias_col(gb2, 2)]
    return np.ascontiguousarray(np.concatenate(cols, 1))  # [128, 8] fp32


BB1, BB2, BB3, BSB1, BSB2, BSB3, BGB1, BGB2 = range(8)


def _build_nc(sel):
    import concourse.bacc as bacc
    import concourse.mybir as mybir
    from concourse.tile import TileContext

    f32 = mybir.dt.float32
    bf16 = mybir.dt.bfloat16
    Relu = mybir.ActivationFunctionType.Relu
    Ident = mybir.ActivationFunctionType.Identity
    add_op = mybir.AluOpType.add
    max_op = mybir.AluOpType.max

    u_sel = sel % 8
    g_sel = u_sel // 2          # z3 pair holding the sel agent
    si = sel // 8               # partition group of sel agent's features

    nc = bacc.Bacc("TRN2", target_bir_lowering=False, debug=False,
                   num_devices=N_CORES)
    x_ext = nc.dram_tensor("xt", [128, B_C], bf16, kind="ExternalInput").ap()
    w_ext = nc.dram_tensor("wpack", [128, WCOLS], bf16, kind="ExternalInput").ap()
    b_ext = nc.dram_tensor("bias", [128, 8], f32, kind="ExternalInput").ap()
    o_ext = nc.dram_tensor("out", [2, B_C], f32, kind="ExternalOutput").ap()

    with TileContext(nc) as tc:
        with (
            tc.tile_pool(name="const", bufs=1) as cpool,
            tc.tile_pool(name="xin", bufs=8) as xpool,
            tc.tile_pool(name="h", bufs=6) as hpool,
            tc.tile_pool(name="z3s", bufs=20) as z3pool,
            tc.tile_pool(name="gl", bufs=2) as glpool,
            tc.tile_pool(name="qs", bufs=4) as qpool,
            tc.tile_pool(name="zp", bufs=2, space="PSUM") as zpool,
            tc.tile_pool(name="z3p", bufs=2, space="PSUM") as z3ppool,
            tc.tile_pool(name="gz", bufs=1, space="PSUM") as gzpool,
            tc.tile_pool(name="gm", bufs=1, space="PSUM") as gmpool,
        ):
            W = cpool.tile([128, WCOLS], bf16, name="W")
            nc.sync.dma_start(out=W[:], in_=w_ext[:])
            BI = cpool.tile([128, 8], f32, name="BI")
            nc.sync.dma_start(out=BI[:], in_=b_ext[:])

            ECYC = [0]

            def evac(dst, src, bcol, func, lo=0, size=128, eng=None):
                b = BI[lo:lo + size, bcol:bcol + 1]
                if eng is None:
                    eng = ECYC[0] % 2
                    ECYC[0] += 1
                if eng == 0:
                    if func == "relu":
                        nc.scalar.activation(dst, src, Relu, bias=b)
                    else:
                        nc.scalar.activation(dst, src, Ident, bias=b)
                else:
                    if func == "relu":
                        nc.vector.tensor_scalar(dst, src, b, 0.0, add_op, max_op)
                    else:
                        nc.vector.tensor_scalar_add(dst, src, b)

            xin_t = {}
            h1_t = {}
            h2_t = {}
            z3s_all = {}
            bsh3_g = {}
            zg_t = {}
            qsb_t = {}

            def emit_xin(c):
                b0 = c * NCH
                xin = xpool.tile([128, NCH], bf16, tag="xin", name=f"xin{c}")
                nc.sync.dma_start(out=xin[:], in_=x_ext[:, b0:b0 + NCH])
                xin_t[c] = xin

            def emit_L1_pair(c, p):
                xin = xin_t[c]
                z1 = zpool.tile([128, 2 * NCH], f32, tag="z", name=f"z1_{c}_{p}")
                for h in range(2):
                    u = 2 * p + h
                    nc.tensor.matmul(
                        z1[:, NCH * h:NCH * h + NCH],
                        W[:, OW1 + 128 * u:OW1 + 128 * u + 128],
                        xin[:], start=True, stop=True)
                h1 = hpool.tile([128, 2 * NCH], bf16, tag="h1", name=f"h1_{c}_{p}")
                evac(h1[:], z1[:], BB1, "relu")
                h1_t[(c, p)] = h1

            def emit_L2_pair(c, p):
                h1 = h1_t.pop((c, p))
                z2 = zpool.tile([128, 2 * NCH], f32, tag="z", name=f"z2_{c}_{p}")
                for h in range(2):
                    nc.tensor.matmul(
                        z2[:, NCH * h:NCH * h + NCH],
                        W[:, OW2:OW2 + 128],
                        h1[:, NCH * h:NCH * h + NCH], start=True, stop=True)
                h2 = hpool.tile([128, 2 * NCH], bf16, tag="h2", name=f"h2_{c}_{p}")
                evac(h2[:], z2[:], BB2, "relu")
                h2_t[(c, p)] = h2

            def emit_L3_pair(c, p):
                # z3 pair tile: part 64c' + 16k + t <-> agent 8k + 2p + c'
                z3 = z3ppool.tile([128, NCH], f32, tag="z3",
                                  name=f"z3_{c}_{p}")
                hp = h2_t[(c, p)]
                for h in range(2):
                    nc.tensor.matmul(
                        z3[64 * h:64 * h + 64, :],
                        W[:, OW3:OW3 + 64],
                        hp[:, NCH * h:NCH * h + NCH],
                        start=True, stop=True)
                del h2_t[(c, p)]
                z3sb = z3pool.tile([128, NCH], bf16, tag="z3s",
                                   name=f"z3s_{c}_{p}")
                evac(z3sb[:], z3[:], BB3, "relu")
                z3s_all.setdefault(c, {})[p] = z3sb

            def emit_selb(grp):
                # batched sel-MLP for chunks 4g..4g+3: chunk stripe = 32*(c%4)
                bz1 = gmpool.tile([128, NCH], f32, tag="g", name=f"bz1_{grp}")
                for cl in range(4):
                    cc = 4 * grp + cl
                    nc.tensor.matmul(
                        bz1[32 * cl:32 * cl + 32, :],
                        W[32 * si:32 * si + 32, OSW1:OSW1 + 32],
                        xin_t[cc][32 * si:32 * si + 32, :],
                        start=True, stop=True,
                        tile_position=(32 * si, 32 * cl))
                bsh1 = glpool.tile([128, NCH], bf16, tag="sh1", name=f"bsh1_{grp}")
                evac(bsh1[:], bz1[:], BSB1, "relu")
                bz2 = gmpool.tile([128, NCH], f32, tag="g", name=f"bz2_{grp}")
                for cl in range(4):
                    nc.tensor.matmul(
                        bz2[32 * cl:32 * cl + 32, :],
                        W[32 * cl:32 * cl + 32, OSW2:OSW2 + 32],
                        bsh1[32 * cl:32 * cl + 32, :],
                        start=True, stop=True,
                        tile_position=(32 * cl, 32 * cl))
                bsh2 = glpool.tile([128, NCH], bf16, tag="sh2", name=f"bsh2_{grp}")
                evac(bsh2[:], bz2[:], BSB2, "relu")
                bz3 = gmpool.tile([128, NCH], f32, tag="g", name=f"bz3_{grp}")
                for cl in range(4):
                    nc.tensor.matmul(
                        bz3[32 * cl:32 * cl + 32, :],
                        W[32 * cl:32 * cl + 32, OSW3:OSW3 + 32],
                        bsh2[32 * cl:32 * cl + 32, :],
                        start=True, stop=True,
                        tile_position=(32 * cl, 32 * cl))
                bsh3 = glpool.tile([128, NCH], bf16, tag="sh3", name=f"bsh3_{grp}")
                evac(bsh3[:], bz3[:], BSB3, "relu")
                bsh3_g[grp] = bsh3

            def emit_gsum_mm(c, step):
                # stripe cl = c%4 of the group tile zg_{c//4}
                G, cl = c // 4, c % 4
                if step == 0 and cl == 0:
                    zg_t[G] = gzpool.tile([128, NCH], f32, tag="zg",
                                          name=f"zg_{G}")
                zg = zg_t[G]
                excl = (step == g_sel)
                oo = OGSUME if excl else OGSUMF
                nc.tensor.matmul(
                    zg[32 * cl:32 * cl + 32, :], W[:, oo:oo + 32],
                    z3s_all[c][step][:, :],
                    start=(step == 0), stop=False,
                    tile_position=(0, 32 * cl))

            def emit_gsel_mm(c):
                G, cl = c // 4, c % 4
                bsh3 = bsh3_g[c // 4]
                nc.tensor.matmul(
                    zg_t[G][32 * cl:32 * cl + 32, :],
                    W[32 * cl:32 * cl + 16, OGSEL:OGSEL + 32],
                    bsh3[32 * cl:32 * cl + 16, :],
                    start=False, stop=True, tile_position=(32 * cl, 32 * cl))
                del z3s_all[c]

            def emit_group_fin(G):
                # all 4 stripes of zg_G accumulated: one hg evac, 4 qp MMs,
                # one q evac into SBUF staging, 4 output DMAs
                zg = zg_t.pop(G)
                hg = glpool.tile([128, NCH], bf16, tag="hg", name=f"hg_{G}")
                evac(hg[:], zg[:], BGB1, "relu")
                qp = gmpool.tile([128, NCH], f32, tag="g", name=f"qp_{G}")
                for cl in range(4):
                    nc.tensor.matmul(
                        qp[32 * cl:32 * cl + 32, :],
                        W[32 * cl:32 * cl + 32, OGW2:OGW2 + 32],
                        hg[32 * cl:32 * cl + 32, :],
                        start=True, stop=True,
                        tile_position=(32 * cl, 32 * cl))
                qsb = qpool.tile([128, NCH], f32, tag="q", name=f"qsb_{G}")
                evac(qsb[:], qp[:], BGB2, "add")
                qsb_t[G] = qsb
                for cl in range(4):
                    b0 = (4 * G + cl) * NCH
                    nc.sync.dma_start(
                        out=o_ext[0:2, b0:b0 + NCH],
                        in_=qsb[32 * cl:32 * cl + 2, :])

            LAG = 4  # tail(c) needs bsh3 of group c//4 (ready in chunk 4g+3)
            for c in range(2):
                emit_xin(c)
            for c in range(CHUNKS):
                if c + 2 < CHUNKS:
                    emit_xin(c + 2)
                tail = c - LAG if c >= LAG else None
                emit_L1_pair(c, 0)
                emit_L1_pair(c, 1)
                if tail is not None:
                    emit_gsum_mm(tail, 0)
                emit_L1_pair(c, 2)
                emit_L1_pair(c, 3)
                if tail is not None:
                    emit_gsum_mm(tail, 1)
                emit_L2_pair(c, 0)
                emit_L2_pair(c, 1)
                if tail is not None:
                    emit_gsum_mm(tail, 2)
                emit_L2_pair(c, 2)
                emit_L2_pair(c, 3)
                if tail is not None:
                    emit_gsum_mm(tail, 3)
                    emit_gsel_mm(tail)
                emit_L3_pair(c, 0)
                emit_L3_pair(c, 1)
                if tail is not None and tail % 4 == 3:
                    emit_group_fin(tail // 4)
                emit_L3_pair(c, 2)
                emit_L3_pair(c, 3)
                if c % 4 == 3:
                    emit_selb(c // 4)
            # drain: remaining LAG tails (one full group), stripes
            # col-parallel so the 4 accumulation chains interleave on the PE
            t0 = CHUNKS - LAG
            for st in range(4):
                for t in range(t0, CHUNKS):
                    emit_gsum_mm(t, st)
            for t in range(t0, CHUNKS):
                emit_gsel_mm(t)
            emit_group_fin(t0 // 4)
    nc.compile()
    return nc


def kernel(**inputs):
    x = _f32(inputs["joint_state_actions"])
    sel = int(inputs["selected_agent_idx"])

    wpack = _build_wpack(
        sel, inputs["oW1"], inputs["oW2"], inputs["oW3"],
        inputs["sW1"], inputs["sW2"], inputs["sW3"],
        inputs["gW1"], inputs["gW2"])
    bias = _build_bias(
        inputs["ob1"], inputs["ob2"], inputs["ob3"],
        inputs["sb1"], inputs["sb2"], inputs["sb3"],
        inputs["gb1"], inputs["gb2"])

    if sel not in _COMPILED:
        _COMPILED[sel] = _build_nc(sel)
    nc = _COMPILED[sel]

    from concourse.bass_utils import run_bass_kernel_spmd
    shards = [np.ascontiguousarray(x[i * B_C:(i + 1) * B_C].T.astype(BF16))
              for i in range(N_CORES)]
    in_maps = [{"xt": s, "wpack": wpack, "bias": bias} for s in shards]
    import os
    trace = bool(int(os.environ.get("KERNEL_TRACE", "0")))
    res = run_bass_kernel_spmd(nc, in_maps, list(range(N_CORES)),
                               trace=trace)
    global LAST_RESULT
    LAST_RESULT = res

    q01 = np.concatenate([res.results[i]["out"] for i in range(N_CORES)],
                         axis=1)
    act = np.clip(x[:, 4 * sel + 3].astype(np.int32), 0, 1)
    out = np.where(act == 0, q01[0], q01[1]).astype(np.float32)
    return out[:, None]
